# revision 10
# baseline (speedup 1.0000x reference)
"""Trainium2 Bass kernel for nn_DynaResidualBlockX (hypernet + per-sample 1x1 conv residual block).

Strategy (8 NeuronCores):
  - Hypernet `ks = lat @ W.T + b` is sharded by W *rows*: each core computes
    1/8 of the per-sample conv kernels for ALL 16 samples (reads 1/8 of W).
  - W rows are permuted + padded on the host ("W2" layout) so the hypernet
    matmul directly produces each conv-weight matrix in the transposed [K, M]
    layout the tensor engine wants, 128-row-aligned per output column.
  - An AllToAll exchanges per-sample kernel slices so core c ends up with the
    full kernel set for its 2 samples (samples 2c, 2c+1).
  - Conv phase: per-sample 1x1 convs (= matmuls over the 16384 pixels),
    relu+bias fused on DVE/ACT, batch-sharded 2 samples/core.
"""

import sys

if "/opt/trn_rl_repo" not in sys.path:
    sys.path.insert(0, "/opt/trn_rl_repo")

import numpy as np

# ---------------- problem constants (hardcoded per contract) ----------------
B, FIN, FOUT, FH, LAT = 16, 64, 64, 128, 512
HWP = 128 * 128  # pixels per image
NCORE, BC = 8, 2  # cores, samples per core
TILES, TPC = 520, 65  # 128-row ks tiles total / per core
KT2, KS = TILES * 128, TPC * 128
JP = 2048  # conv pixel chunk
NJ = HWP // JP  # 8 chunks
NT = JP // 512  # 4 matmuls (N=512) per chunk
WCH = 13  # ks tiles per W DMA chunk (65 = 5 * 13)
NWCH = TPC // WCH  # 5

# tile bases in the W2 layout
T_KIN, T_MIDA, T_MIDB, T_KOUT, T_KSH = 0, 128, 256, 384, 448
T_BMIDA, T_BMIDB = 512, 513

S128 = 1.0 / np.sqrt(128.0)
S64 = 1.0 / 8.0


def _build_w2b2(W, b):
    """Permute/pad/scale hypernet weights into the device tile layout.

    Row r = t*128 + p of W2 produces ks-tile t, partition p. Returns
    W2 [KT2, LAT] and b2 [KT2].
    """
    r = np.arange(KT2)
    t, p = r >> 7, r & 127
    src = np.full(KT2, -1, np.int64)
    scale = np.ones(KT2, np.float32)

    m = (t < 128) & (p < 64)
    src[m] = t[m] * 64 + p[m]
    scale[m] = S128
    m = (t < 128) & (p == 64)  # b_in folded into the ones-channel row
    src[m] = 53248 + t[m]
    m = (t >= 128) & (t < 256)
    src[m] = 8192 + (t[m] - 128) * 128 + p[m]
    scale[m] = S128
    m = (t >= 256) & (t < 384)
    src[m] = 24576 + (t[m] - 256) * 128 + p[m]
    scale[m] = S128
    m = (t >= 384) & (t < 448)
    src[m] = 40960 + (t[m] - 384) * 128 + p[m]
    scale[m] = S64
    m = (t >= 448) & (t < 512) & (p < 64)
    src[m] = 49152 + (t[m] - 448) * 64 + p[m]
    scale[m] = S64
    m_bos = (t >= 448) & (t < 512) & (p == 64)  # b_out + b_short combined
    src[m_bos] = 53632 + (t[m_bos] - 448)
    m = t == T_BMIDA
    src[m] = 53376 + p[m]
    m = t == T_BMIDB
    src[m] = 53504 + p[m]

    W2 = np.zeros((KT2, LAT), np.float32)
    b2 = np.zeros(KT2, np.float32)
    v = src >= 0
    W2[v] = W[src[v]] * scale[v][:, None]
    b2[v] = b[src[v]] * scale[v]
    # add the b_short rows into the combined bias row
    W2[m_bos] += W[53696 + (t[m_bos] - 448)]
    b2[m_bos] += b[53696 + (t[m_bos] - 448)]
    return W2, b2


def _host_inputs(x, lat, W, b):
    """Build the 8 per-core input maps (pure layout work, no math)."""
    x = np.ascontiguousarray(x, np.float32).reshape(B, FIN, HWP)
    lat = np.ascontiguousarray(lat, np.float32)
    W2, b2 = _build_w2b2(np.asarray(W, np.float32), np.asarray(b, np.float32))

    # lat.T in chunk layout: latt[q, l*16 + b] = lat[b, l*128+q]
    latt = np.ascontiguousarray(
        lat.T.reshape(4, 128, 16).transpose(1, 0, 2).reshape(128, 64)
    )
    # bias lookup duplicated per local sample col: b2dup[p, t*2+b'] = b2[t*128+p]
    b2dup = np.ascontiguousarray(
        np.repeat(b2.reshape(TILES, 128).T[:, :, None], 2, axis=2).reshape(128, TILES * 2)
    )
    # x with a ones channel appended (drives the bias rows of kin/kshort)
    xs_all = np.concatenate(
        [x, np.ones((B, 1, HWP), np.float32)], axis=1
    )  # [B, 65, HWP]

    in_maps = []
    for c in range(NCORE):
        shard = W2[c * KS : (c + 1) * KS]  # [KS, LAT], row = tl*128 + m
        # wmain[q, tl, l, m] = W2[(c*KS)+tl*128+m, l*128+q]
        wmain = np.ascontiguousarray(
            shard.reshape(TPC, 128, 4, 128).transpose(3, 0, 2, 1).reshape(128, TPC * 4 * 128)
        )
        in_maps.append(
            {
                "wmain": wmain,
                "latt": latt,
                "b2dup": b2dup,
                "xs": np.ascontiguousarray(xs_all[c * BC : (c + 1) * BC]),
            }
        )
    return in_maps


def emulate(x, lat, W, b):
    """Numpy emulation of the exact device dataflow (for layout validation)."""
    x = np.asarray(x, np.float32).reshape(B, FIN, HWP)
    W2, b2 = _build_w2b2(np.asarray(W, np.float32), np.asarray(b, np.float32))
    ksT = W2 @ np.asarray(lat, np.float32).T + b2[:, None]  # [KT2, 16]
    v = ksT.reshape(TILES, 128, B)  # [t, p, b]
    out = np.zeros((B, FOUT, HWP), np.float32)
    for bi in range(B):
        xb = np.concatenate([x[bi], np.ones((1, HWP), np.float32)], axis=0)  # [65, HWP]
        a_in = v[T_KIN : T_KIN + 128, 0:65, bi].T  # lhsT [65, 128]
        h1 = np.maximum(a_in.T @ xb, 0.0)
        a_ma = v[T_MIDA : T_MIDA + 128, :, bi].T  # [128, 128]
        bma = v[T_BMIDA, :, bi][:, None]
        h2 = np.maximum(a_ma.T @ h1 + bma, 0.0)
        a_mb = v[T_MIDB : T_MIDB + 128, :, bi].T
        bmb = v[T_BMIDB, :, bi][:, None]
        h3 = np.maximum(a_mb.T @ h2 + bmb, 0.0)
        a_out = v[T_KOUT : T_KOUT + 64, :, bi].T  # [128, 64]
        a_sh = v[T_KSH : T_KSH + 64, 0:65, bi].T  # [65, 64]
        out[bi] = a_out.T @ h3 + a_sh.T @ xb
    return out.reshape(B, FOUT, 128, 128)


# ---------------------------- bass program ----------------------------------

def _build_nc():
    import concourse.bass as bass
    import concourse.tile as tile
    from concourse import bacc, mybir

    F32, F32R = mybir.dt.float32, mybir.dt.float32r
    AF = mybir.ActivationFunctionType
    ALU = mybir.AluOpType

    nc = bacc.Bacc("TRN2", target_bir_lowering=False, debug=False, num_devices=NCORE)

    wmain = nc.dram_tensor("wmain", [128, TPC * 4 * 128], F32R, kind="ExternalInput")
    latt = nc.dram_tensor("latt", [128, 64], F32R, kind="ExternalInput")
    b2dup = nc.dram_tensor("b2dup", [128, TILES * 2], F32R, kind="ExternalInput")
    xs = nc.dram_tensor("xs", [BC, FIN + 1, HWP], F32R, kind="ExternalInput")
    outd = nc.dram_tensor("out", [BC, FOUT, HWP], F32, kind="ExternalOutput")

    with tile.TileContext(nc) as tc:
        with (
            tc.tile_pool(name="persist", bufs=1) as pp,
            tc.tile_pool(name="wpool", bufs=2) as wp,
            tc.tile_pool(name="conv", bufs=2) as cp,
            tc.tile_pool(name="ps", bufs=2, space="PSUM") as psp,
            tc.tile_pool(name="dram", bufs=1, space="DRAM") as dp,
        ):
            latt_sb = pp.tile([128, 64], F32R)
            nc.sync.dma_start(latt_sb[:], latt[:])
            b2_sb = pp.tile([128, TILES * 2], F32R)
            nc.sync.dma_start(b2_sb[:], b2dup[:])

            # ---- hypernet: ks tiles for this core's W shard, all 16 samples
            # ks_sh column order: d (dest core) major, then tile, then local sample
            ks_sh = pp.tile([128, TPC * 16], F32R)
            ks_sh_v = ks_sh.rearrange("p (d t s) -> p d t s", d=NCORE, t=TPC, s=BC)
            wmv = wmain[:].rearrange("p (t l m) -> p t l m", t=TPC, l=4, m=128)
            for ch in range(NWCH):
                wt = wp.tile([128, WCH * 4 * 128], F32R, tag="wt")
                nc.sync.dma_start(
                    wt[:], wmv[:, ch * WCH : (ch + 1) * WCH].rearrange("p t l m -> p (t l m)")
                )
                wtv = wt.rearrange("p (t l m) -> p t l m", t=WCH, l=4)
                pk = psp.tile([128, WCH * 16], F32, tag="ps")
                for tl in range(WCH):
                    for l in range(4):
                        nc.tensor.matmul(
                            pk[:, tl * 16 : (tl + 1) * 16],
                            wtv[:, tl, l, :],
                            latt_sb[:, l * 16 : (l + 1) * 16],
                            start=(l == 0),
                            stop=(l == 3),
                        )
                # permuting copy PSUM -> SBUF: col tl*16 + (d*2+s) -> (d, tile, s)
                pkv = pk.rearrange("p (t d s) -> p d t s", t=WCH, d=NCORE, s=BC)
                nc.vector.tensor_copy(
                    ks_sh_v[:, :, ch * WCH : (ch + 1) * WCH, :], pkv
                )

            # ---- AllToAll: core c receives the full kernel set for its samples
            cc_in = dp.tile([NCORE, 128, TPC * BC], F32R)
            nc.sync.dma_start(
                cc_in[:].rearrange("d p r -> p d r"),
                ks_sh_v.rearrange("p d t s -> p d (t s)"),
            )
            cc_out = dp.tile([NCORE, 128, TPC * BC], F32R)
            nc.gpsimd.collective_compute(
                "AllToAll",
                ALU.bypass,
                replica_groups=[list(range(NCORE))],
                ins=[cc_in.opt()],
                outs=[cc_out.opt()],
            )
            ksraw = pp.tile([128, TILES * BC], F32R)
            nc.sync.dma_start(
                ksraw.rearrange("p (c r) -> p c r", c=NCORE),
                cc_out[:].rearrange("c p r -> p c r"),
            )
            ksall = pp.tile([128, TILES * BC], F32R)
            nc.vector.tensor_tensor(ksall[:], ksraw[:], b2_sb[:], op=ALU.add)
            ksv = ksall.rearrange("p (t s) -> p t s", t=TILES, s=BC)

            # ---- conv: 2 samples x 8 pixel chunks
            for j in range(NJ):
                jsl = slice(j * JP, (j + 1) * JP)
                xcs = []
                for s in range(BC):
                    xc = cp.tile([FIN + 1, JP], F32R, tag="xc", bufs=4)
                    nc.sync.dma_start(xc[:], xs[s, :, jsl])
                    xcs.append(xc)
                for s in range(BC):
                    xc = xcs[s]
                    ph1 = psp.tile([128, JP], F32, tag="ps")
                    for n in range(NT):
                        nsl = slice(n * 512, (n + 1) * 512)
                        nc.tensor.matmul(
                            ph1[:, nsl], ksv[0:65, T_KIN : T_KIN + 128, s], xc[:, nsl],
                            start=True, stop=True,
                        )
                    h1 = cp.tile([128, JP], F32R, tag="h", bufs=4)
                    nc.vector.tensor_scalar_max(h1[:, 0:1024], ph1[:, 0:1024], 0.0)
                    nc.scalar.activation(h1[:, 1024:JP], ph1[:, 1024:JP], AF.Relu)

                    ph2 = psp.tile([128, JP], F32, tag="ps")
                    for n in range(NT):
                        nsl = slice(n * 512, (n + 1) * 512)
                        nc.tensor.matmul(
                            ph2[:, nsl], ksv[:, T_MIDA : T_MIDA + 128, s], h1[:, nsl],
                            start=True, stop=True,
                        )
                    h2 = cp.tile([128, JP], F32R, tag="h", bufs=4)
                    bma = ksv[:, T_BMIDA, s : s + 1].bitcast(F32)
                    nc.vector.tensor_scalar(
                        h2[:, 0:1024], ph2[:, 0:1024], bma, 0.0, op0=ALU.add, op1=ALU.max
                    )
                    nc.scalar.activation(h2[:, 1024:JP], ph2[:, 1024:JP], AF.Relu, bias=bma)

                    ph3 = psp.tile([128, JP], F32, tag="ps")
                    for n in range(NT):
                        nsl = slice(n * 512, (n + 1) * 512)
                        nc.tensor.matmul(
                            ph3[:, nsl], ksv[:, T_MIDB : T_MIDB + 128, s], h2[:, nsl],
                            start=True, stop=True,
                        )
                    h3 = cp.tile([128, JP], F32R, tag="h", bufs=4)
                    bmb = ksv[:, T_BMIDB, s : s + 1].bitcast(F32)
                    nc.vector.tensor_scalar(
                        h3[:, 0:1024], ph3[:, 0:1024], bmb, 0.0, op0=ALU.add, op1=ALU.max
                    )
                    nc.scalar.activation(h3[:, 1024:JP], ph3[:, 1024:JP], AF.Relu, bias=bmb)

                    # out layer: k_out @ h3 + k_short' @ [x; 1]  (bias via ones row)
                    po = psp.tile([64, JP], F32, tag="ps")
                    for n in range(NT):
                        nsl = slice(n * 512, (n + 1) * 512)
                        nc.tensor.matmul(
                            po[:, nsl],
                            ksv[:, T_KOUT : T_KOUT + 64, s],
                            h3[:, nsl],
                            start=True, stop=False,
                        )
                        nc.tensor.matmul(
                            po[:, nsl],
                            ksv[0:65, T_KSH : T_KSH + 64, s],
                            xc[:, nsl],
                            start=False, stop=True,
                        )
                    oc = cp.tile([64, JP], F32, tag="oc", bufs=4)
                    nc.vector.tensor_copy(oc[:, 0:1024], po[:, 0:1024])
                    nc.scalar.activation(oc[:, 1024:JP], po[:, 1024:JP], AF.Copy)
                    nc.sync.dma_start(outd[s, :, jsl], oc[:])

    nc.compile()
    return nc


_NC_CACHE = None


def kernel(x, lat, W, b):
    from concourse.bass_utils import run_bass_kernel_spmd

    global _NC_CACHE
    if _NC_CACHE is None:
        _NC_CACHE = _build_nc()
    nc = _NC_CACHE
    in_maps = _host_inputs(x, lat, W, b)
    res = run_bass_kernel_spmd(nc, in_maps, core_ids=list(range(NCORE)))
    out = np.concatenate([res.results[c]["out"] for c in range(NCORE)], axis=0)
    return np.ascontiguousarray(out.reshape(B, FOUT, 128, 128))


# revision 11
# speedup vs baseline: 1.1961x; 1.1961x over previous
"""Trainium2 Bass kernel for nn_DynaResidualBlockX (hypernet + per-sample 1x1 conv residual block).

Strategy (8 NeuronCores):
  - Hypernet `ks = lat @ W.T + b` is sharded by W *rows*: each core computes
    1/8 of the per-sample conv kernels for ALL 16 samples (reads 1/8 of W).
  - W rows are permuted + padded on the host ("W2" layout) so the hypernet
    matmul directly produces each conv-weight matrix in the transposed [K, M]
    layout the tensor engine wants, 128-row-aligned per output column.
  - An AllToAll exchanges per-sample kernel slices so core c ends up with the
    full kernel set for its 2 samples (samples 2c, 2c+1).
  - Conv phase: per-sample 1x1 convs (= matmuls over the 16384 pixels),
    relu+bias fused, alternating between DVE and ACT engines per psum tile.
"""

import os
import sys

if "/opt/trn_rl_repo" not in sys.path:
    sys.path.insert(0, "/opt/trn_rl_repo")

import numpy as np

# ---------------- problem constants (hardcoded per contract) ----------------
B, FIN, FOUT, FH, LAT = 16, 64, 64, 128, 512
HWP = 128 * 128  # pixels per image
NCORE, BC = 8, 2  # cores, samples per core
TILES, TPC = 520, 65  # 128-row ks tiles total / per core
KT2, KS = TILES * 128, TPC * 128
JP = 2048  # conv pixel chunk
NJ = HWP // JP  # 8 chunks
NP = 1024  # psum tile / act-op granularity
WCH = 13  # ks tiles per W DMA chunk (65 = 5 * 13)
NWCH = TPC // WCH  # 5

# dtype mode: "bf16" (fast, ~3e-3 abs-rel err) or "f32r" (~2e-4 err)
DT_MODE = os.environ.get("KERNEL_DT", "f32r")

# tile bases in the W2 layout
T_KIN, T_MIDA, T_MIDB, T_KOUT, T_KSH = 0, 128, 256, 384, 448
T_BMIDA, T_BMIDB = 512, 513

S128 = 1.0 / np.sqrt(128.0)
S64 = 1.0 / 8.0


def _np_dt():
    if DT_MODE == "bf16":
        import ml_dtypes

        return ml_dtypes.bfloat16
    return np.float32


def _build_w2b2(W, b):
    """Permute/pad/scale hypernet weights into the device tile layout.

    Row r = t*128 + p of W2 produces ks-tile t, partition p. Returns
    W2 [KT2, LAT] and b2 [KT2].
    """
    r = np.arange(KT2)
    t, p = r >> 7, r & 127
    src = np.full(KT2, -1, np.int64)
    scale = np.ones(KT2, np.float32)

    m = (t < 128) & (p < 64)
    src[m] = t[m] * 64 + p[m]
    scale[m] = S128
    m = (t < 128) & (p == 64)  # b_in folded into the ones-channel row
    src[m] = 53248 + t[m]
    m = (t >= 128) & (t < 256)
    src[m] = 8192 + (t[m] - 128) * 128 + p[m]
    scale[m] = S128
    m = (t >= 256) & (t < 384)
    src[m] = 24576 + (t[m] - 256) * 128 + p[m]
    scale[m] = S128
    m = (t >= 384) & (t < 448)
    src[m] = 40960 + (t[m] - 384) * 128 + p[m]
    scale[m] = S64
    m = (t >= 448) & (t < 512) & (p < 64)
    src[m] = 49152 + (t[m] - 448) * 64 + p[m]
    scale[m] = S64
    m_bos = (t >= 448) & (t < 512) & (p == 64)  # b_out + b_short combined
    src[m_bos] = 53632 + (t[m_bos] - 448)
    m = t == T_BMIDA
    src[m] = 53376 + p[m]
    m = t == T_BMIDB
    src[m] = 53504 + p[m]

    W2 = np.zeros((KT2, LAT), np.float32)
    b2 = np.zeros(KT2, np.float32)
    v = src >= 0
    W2[v] = W[src[v]] * scale[v][:, None]
    b2[v] = b[src[v]] * scale[v]
    # add the b_short rows into the combined bias row
    W2[m_bos] += W[53696 + (t[m_bos] - 448)]
    b2[m_bos] += b[53696 + (t[m_bos] - 448)]
    return W2, b2


def _host_inputs(x, lat, W, b):
    """Build the 8 per-core input maps (pure layout work, no math)."""
    ndt = _np_dt()
    x = np.ascontiguousarray(x, np.float32).reshape(B, FIN, HWP)
    lat = np.ascontiguousarray(lat, np.float32)
    W2, b2 = _build_w2b2(np.asarray(W, np.float32), np.asarray(b, np.float32))

    # lat.T in chunk layout: latt[q, l*16 + b] = lat[b, l*128+q]
    latt = np.ascontiguousarray(
        lat.T.reshape(4, 128, 16).transpose(1, 0, 2).reshape(128, 64).astype(ndt)
    )
    # bias lookup duplicated per local sample col: b2dup[p, t*2+b'] = b2[t*128+p]
    b2dup = np.ascontiguousarray(
        np.repeat(b2.reshape(TILES, 128).T[:, :, None], 2, axis=2)
        .reshape(128, TILES * 2)
        .astype(ndt)
    )
    # x with a ones channel appended (drives the bias rows of kin/kshort)
    xs_all = np.concatenate([x, np.ones((B, 1, HWP), np.float32)], axis=1).astype(ndt)

    in_maps = []
    for c in range(NCORE):
        shard = W2[c * KS : (c + 1) * KS]  # [KS, LAT], row = tl*128 + m
        # wmain[q, tl, l, m] = W2[(c*KS)+tl*128+m, l*128+q]
        wmain = np.ascontiguousarray(
            shard.reshape(TPC, 128, 4, 128)
            .transpose(3, 0, 2, 1)
            .reshape(128, TPC * 4 * 128)
            .astype(ndt)
        )
        in_maps.append(
            {
                "wmain": wmain,
                "latt": latt,
                "b2dup": b2dup,
                "xs": np.ascontiguousarray(xs_all[c * BC : (c + 1) * BC]),
            }
        )
    return in_maps


def emulate(x, lat, W, b):
    """Numpy emulation of the exact device dataflow (for layout validation)."""
    x = np.asarray(x, np.float32).reshape(B, FIN, HWP)
    W2, b2 = _build_w2b2(np.asarray(W, np.float32), np.asarray(b, np.float32))
    ksT = W2 @ np.asarray(lat, np.float32).T + b2[:, None]  # [KT2, 16]
    v = ksT.reshape(TILES, 128, B)  # [t, p, b]
    out = np.zeros((B, FOUT, HWP), np.float32)
    for bi in range(B):
        xb = np.concatenate([x[bi], np.ones((1, HWP), np.float32)], axis=0)  # [65, HWP]
        a_in = v[T_KIN : T_KIN + 128, 0:65, bi].T  # lhsT [65, 128]
        h1 = np.maximum(a_in.T @ xb, 0.0)
        a_ma = v[T_MIDA : T_MIDA + 128, :, bi].T  # [128, 128]
        bma = v[T_BMIDA, :, bi][:, None]
        h2 = np.maximum(a_ma.T @ h1 + bma, 0.0)
        a_mb = v[T_MIDB : T_MIDB + 128, :, bi].T
        bmb = v[T_BMIDB, :, bi][:, None]
        h3 = np.maximum(a_mb.T @ h2 + bmb, 0.0)
        a_out = v[T_KOUT : T_KOUT + 64, :, bi].T  # [128, 64]
        a_sh = v[T_KSH : T_KSH + 64, 0:65, bi].T  # [65, 64]
        out[bi] = a_out.T @ h3 + a_sh.T @ xb
    return out.reshape(B, FOUT, 128, 128)


# ---------------------------- bass program ----------------------------------

def _build_nc():
    import concourse.bass as bass
    import concourse.tile as tile
    from concourse import bacc, mybir

    F32 = mybir.dt.float32
    DT = mybir.dt.bfloat16 if DT_MODE == "bf16" else mybir.dt.float32r
    MMN = NP if DT_MODE == "bf16" else 512  # moving free-dim per matmul
    AF = mybir.ActivationFunctionType
    ALU = mybir.AluOpType

    nc = bacc.Bacc("TRN2", target_bir_lowering=False, debug=False, num_devices=NCORE)

    wmain = nc.dram_tensor("wmain", [128, TPC * 4 * 128], DT, kind="ExternalInput")
    latt = nc.dram_tensor("latt", [128, 64], DT, kind="ExternalInput")
    b2dup = nc.dram_tensor("b2dup", [128, TILES * 2], DT, kind="ExternalInput")
    xs = nc.dram_tensor("xs", [BC, FIN + 1, HWP], DT, kind="ExternalInput")
    outd = nc.dram_tensor("out", [BC, FOUT, HWP], F32, kind="ExternalOutput")

    with tile.TileContext(nc) as tc:
        with (
            tc.tile_pool(name="persist", bufs=1) as pp,
            tc.tile_pool(name="wpool", bufs=2) as wp,
            tc.tile_pool(name="conv", bufs=2) as cp,
            tc.tile_pool(name="ps", bufs=4, space="PSUM") as psp,
            tc.tile_pool(name="dram", bufs=1, space="DRAM") as dp,
        ):
            latt_sb = pp.tile([128, 64], DT)
            nc.sync.dma_start(latt_sb[:], latt[:])
            b2_sb = pp.tile([128, TILES * 2], DT)
            nc.sync.dma_start(b2_sb[:], b2dup[:])

            # ---- hypernet: ks tiles for this core's W shard, all 16 samples
            # ks_sh column order: d (dest core) major, then tile, then local sample
            ks_sh = pp.tile([128, TPC * 16], DT)
            ks_sh_v = ks_sh.rearrange("p (d t s) -> p d t s", d=NCORE, t=TPC, s=BC)
            wmv = wmain[:].rearrange("p (t l m) -> p t l m", t=TPC, l=4, m=128)
            for ch in range(NWCH):
                wt = wp.tile([128, WCH * 4 * 128], DT, tag="wt")
                nc.sync.dma_start(
                    wt[:], wmv[:, ch * WCH : (ch + 1) * WCH].rearrange("p t l m -> p (t l m)")
                )
                wtv = wt.rearrange("p (t l m) -> p t l m", t=WCH, l=4)
                pk = psp.tile([128, WCH * 16], F32, tag="ps")
                for tl in range(WCH):
                    for l in range(4):
                        nc.tensor.matmul(
                            pk[:, tl * 16 : (tl + 1) * 16],
                            wtv[:, tl, l, :],
                            latt_sb[:, l * 16 : (l + 1) * 16],
                            start=(l == 0),
                            stop=(l == 3),
                        )
                # permuting copy PSUM -> SBUF: col tl*16 + (d*2+s) -> (d, tile, s)
                pkv = pk.rearrange("p (t d s) -> p d t s", t=WCH, d=NCORE, s=BC)
                nc.vector.tensor_copy(
                    ks_sh_v[:, :, ch * WCH : (ch + 1) * WCH, :], pkv
                )

            # ---- AllToAll: core c receives the full kernel set for its samples
            cc_in = dp.tile([NCORE, 128, TPC * BC], DT)
            nc.sync.dma_start(
                cc_in[:].rearrange("d p r -> p d r"),
                ks_sh_v.rearrange("p d t s -> p d (t s)"),
            )
            cc_out = dp.tile([NCORE, 128, TPC * BC], DT)
            nc.gpsimd.collective_compute(
                "AllToAll",
                ALU.bypass,
                replica_groups=[list(range(NCORE))],
                ins=[cc_in.opt()],
                outs=[cc_out.opt()],
            )
            ksraw = pp.tile([128, TILES * BC], DT)
            nc.sync.dma_start(
                ksraw.rearrange("p (c r) -> p c r", c=NCORE),
                cc_out[:].rearrange("c p r -> p c r"),
            )
            ksall = pp.tile([128, TILES * BC], DT)
            nc.vector.tensor_tensor(ksall[:], ksraw[:], b2_sb[:], op=ALU.add)
            ksv = ksall.rearrange("p (t s) -> p t s", t=TILES, s=BC)
            # f32 per-partition bias vectors for the mid layers
            bias4 = pp.tile([128, 4], F32)
            nc.vector.tensor_copy(bias4[:, 0:2], ksall[:, T_BMIDA * 2 : T_BMIDA * 2 + 2])
            nc.vector.tensor_copy(bias4[:, 2:4], ksall[:, T_BMIDB * 2 : T_BMIDB * 2 + 2])

            # ---- conv: 2 samples x 8 pixel chunks
            ek = 0  # DVE/ACT alternation counter

            def relu_bias(dst, src, bias):
                nonlocal ek
                ek += 1
                if ek % 2 == 0:
                    if bias is None:
                        nc.vector.tensor_scalar_max(dst, src, 0.0)
                    else:
                        nc.vector.tensor_scalar(
                            dst, src, bias, 0.0, op0=ALU.add, op1=ALU.max
                        )
                else:
                    if bias is None:
                        nc.scalar.activation(dst, src, AF.Relu)
                    else:
                        nc.scalar.activation(dst, src, AF.Relu, bias=bias)

            def copy_out(dst, src):
                nonlocal ek
                ek += 1
                if ek % 2 == 0:
                    nc.vector.tensor_copy(dst, src)
                else:
                    nc.scalar.activation(dst, src, AF.Copy)

            def layer(dst_h, lhsT, rhs_tile, bias, kparts):
                for half in range(JP // NP):
                    ph = psp.tile([128, NP], F32, tag="ps", name=f"ph{half}")
                    for n in range(NP // MMN):
                        lo = half * NP + n * MMN
                        nc.tensor.matmul(
                            ph[:, n * MMN : (n + 1) * MMN],
                            lhsT,
                            rhs_tile[0:kparts, lo : lo + MMN],
                            start=True,
                            stop=True,
                        )
                    hsl = slice(half * NP, (half + 1) * NP)
                    relu_bias(dst_h[:, hsl], ph[:, 0:NP], bias)

            for j in range(NJ):
                jsl = slice(j * JP, (j + 1) * JP)
                xcs = []
                for s in range(BC):
                    xc = cp.tile([FIN + 1, JP], DT, tag="xc", bufs=4)
                    nc.scalar.dma_start(xc[:], xs[s, :, jsl])
                    xcs.append(xc)
                for s in range(BC):
                    xc = xcs[s]
                    h1 = cp.tile([128, JP], DT, tag="h", bufs=4)
                    layer(h1, ksv[0:65, T_KIN : T_KIN + 128, s], xc, None, 65)
                    h2 = cp.tile([128, JP], DT, tag="h", bufs=4)
                    layer(h2, ksv[:, T_MIDA : T_MIDA + 128, s], h1, bias4[:, s : s + 1], 128)
                    h3 = cp.tile([128, JP], DT, tag="h", bufs=4)
                    layer(h3, ksv[:, T_MIDB : T_MIDB + 128, s], h2, bias4[:, 2 + s : 3 + s], 128)

                    # out layer: k_out @ h3 + k_short' @ [x; 1]  (bias via ones row)
                    oc = cp.tile([64, JP], F32, tag="oc", bufs=4)
                    for half in range(JP // NP):
                        po = psp.tile([64, NP], F32, tag="ps", name=f"po{half}")
                        for n in range(NP // MMN):
                            lo = half * NP + n * MMN
                            nsl = slice(n * MMN, (n + 1) * MMN)
                            nc.tensor.matmul(
                                po[:, nsl],
                                ksv[:, T_KOUT : T_KOUT + 64, s],
                                h3[:, lo : lo + MMN],
                                start=True,
                                stop=False,
                            )
                            nc.tensor.matmul(
                                po[:, nsl],
                                ksv[0:65, T_KSH : T_KSH + 64, s],
                                xc[:, lo : lo + MMN],
                                start=False,
                                stop=True,
                            )
                        hsl = slice(half * NP, (half + 1) * NP)
                        copy_out(oc[:, hsl], po[:, 0:NP])
                    nc.scalar.dma_start(outd[s, :, jsl], oc[:])

    nc.compile()
    return nc


_NC_CACHE = None


def kernel(x, lat, W, b):
    from concourse.bass_utils import run_bass_kernel_spmd

    global _NC_CACHE
    if _NC_CACHE is None:
        _NC_CACHE = _build_nc()
    nc = _NC_CACHE
    in_maps = _host_inputs(x, lat, W, b)
    res = run_bass_kernel_spmd(nc, in_maps, core_ids=list(range(NCORE)))
    out = np.concatenate([res.results[c]["out"] for c in range(NCORE)], axis=0)
    return np.ascontiguousarray(out.reshape(B, FOUT, 128, 128))


# revision 12
# speedup vs baseline: 1.4998x; 1.2540x over previous
"""Trainium2 Bass kernel for nn_DynaResidualBlockX (hypernet + per-sample 1x1 conv residual block).

Strategy (8 NeuronCores):
  - Hypernet `ks = lat @ W.T + b` is sharded by W *rows*: each core computes
    1/8 of the per-sample conv kernels for ALL 16 samples (reads 1/8 of W).
  - W rows are permuted + padded on the host ("W2" layout) so the hypernet
    matmul directly produces each conv-weight matrix in the transposed [K, M]
    layout the tensor engine wants, 128-row-aligned per output column.
  - An AllToAll exchanges per-sample kernel slices so core c ends up with the
    full kernel set for its 2 samples (samples 2c, 2c+1).
  - Conv phase: per-sample 1x1 convs (= matmuls over the 16384 pixels),
    relu+bias fused, alternating between DVE and ACT engines per psum tile.
"""

import os
import sys

if "/opt/trn_rl_repo" not in sys.path:
    sys.path.insert(0, "/opt/trn_rl_repo")

import numpy as np

# ---------------- problem constants (hardcoded per contract) ----------------
B, FIN, FOUT, FH, LAT = 16, 64, 64, 128, 512
HWP = 128 * 128  # pixels per image
NCORE, BC = 8, 2  # cores, samples per core
TILES, TPC = 520, 65  # 128-row ks tiles total / per core
KT2, KS = TILES * 128, TPC * 128
JP = 2048  # conv pixel chunk
NJ = HWP // JP  # 8 chunks
NP = 1024  # psum tile / act-op granularity
WCH = 13  # ks tiles per W DMA chunk (65 = 5 * 13)
NWCH = TPC // WCH  # 5

# dtype mode: "bf16" (fast, ~3e-3 abs-rel err) or "f32r" (~2e-4 err)
DT_MODE = os.environ.get("KERNEL_DT", "f32r")

# tile bases in the W2 layout
T_KIN, T_MIDA, T_MIDB, T_KOUT, T_KSH = 0, 128, 256, 384, 448
T_BMIDA, T_BMIDB = 512, 513

S128 = 1.0 / np.sqrt(128.0)
S64 = 1.0 / 8.0


def _np_dt():
    if DT_MODE == "bf16":
        import ml_dtypes

        return ml_dtypes.bfloat16
    return np.float32


def _build_w2b2(W, b):
    """Permute/pad/scale hypernet weights into the device tile layout.

    Row r = t*128 + p of W2 produces ks-tile t, partition p. Returns
    W2 [KT2, LAT] and b2 [KT2].
    """
    r = np.arange(KT2)
    t, p = r >> 7, r & 127
    src = np.full(KT2, -1, np.int64)
    scale = np.ones(KT2, np.float32)

    m = (t < 128) & (p < 64)
    src[m] = t[m] * 64 + p[m]
    scale[m] = S128
    m = (t < 128) & (p == 64)  # b_in folded into the ones-channel row
    src[m] = 53248 + t[m]
    m = (t >= 128) & (t < 256)
    src[m] = 8192 + (t[m] - 128) * 128 + p[m]
    scale[m] = S128
    m = (t >= 256) & (t < 384)
    src[m] = 24576 + (t[m] - 256) * 128 + p[m]
    scale[m] = S128
    m = (t >= 384) & (t < 448)
    src[m] = 40960 + (t[m] - 384) * 128 + p[m]
    scale[m] = S64
    m = (t >= 448) & (t < 512) & (p < 64)
    src[m] = 49152 + (t[m] - 448) * 64 + p[m]
    scale[m] = S64
    m_bos = (t >= 448) & (t < 512) & (p == 64)  # b_out + b_short combined
    src[m_bos] = 53632 + (t[m_bos] - 448)
    m = t == T_BMIDA
    src[m] = 53376 + p[m]
    m = t == T_BMIDB
    src[m] = 53504 + p[m]

    W2 = np.zeros((KT2, LAT), np.float32)
    b2 = np.zeros(KT2, np.float32)
    v = src >= 0
    W2[v] = W[src[v]] * scale[v][:, None]
    b2[v] = b[src[v]] * scale[v]
    # add the b_short rows into the combined bias row
    W2[m_bos] += W[53696 + (t[m_bos] - 448)]
    b2[m_bos] += b[53696 + (t[m_bos] - 448)]
    return W2, b2


def _host_inputs(x, lat, W, b):
    """Build the 8 per-core input maps (pure layout work, no math)."""
    ndt = _np_dt()
    x = np.ascontiguousarray(x, np.float32).reshape(B, FIN, HWP)
    lat = np.ascontiguousarray(lat, np.float32)
    W2, b2 = _build_w2b2(np.asarray(W, np.float32), np.asarray(b, np.float32))

    # lat.T in chunk layout: latt[q, l*16 + b] = lat[b, l*128+q]
    latt = np.ascontiguousarray(
        lat.T.reshape(4, 128, 16).transpose(1, 0, 2).reshape(128, 64).astype(ndt)
    )
    # bias lookup duplicated per local sample col: b2dup[p, t*2+b'] = b2[t*128+p]
    b2dup = np.ascontiguousarray(
        np.repeat(b2.reshape(TILES, 128).T[:, :, None], 2, axis=2)
        .reshape(128, TILES * 2)
        .astype(ndt)
    )
    # x with a ones channel appended (drives the bias rows of kin/kshort)
    xs_all = np.concatenate([x, np.ones((B, 1, HWP), np.float32)], axis=1).astype(ndt)

    in_maps = []
    for c in range(NCORE):
        shard = W2[c * KS : (c + 1) * KS]  # [KS, LAT], row = tl*128 + m
        # wmain[q, tl, l, m] = W2[(c*KS)+tl*128+m, l*128+q]
        wmain = np.ascontiguousarray(
            shard.reshape(TPC, 128, 4, 128)
            .transpose(3, 0, 2, 1)
            .reshape(128, TPC * 4 * 128)
            .astype(ndt)
        )
        in_maps.append(
            {
                "wmain": wmain,
                "latt": latt,
                "b2dup": b2dup,
                "xs": np.ascontiguousarray(xs_all[c * BC : (c + 1) * BC]),
            }
        )
    return in_maps


def emulate(x, lat, W, b):
    """Numpy emulation of the exact device dataflow (for layout validation)."""
    x = np.asarray(x, np.float32).reshape(B, FIN, HWP)
    W2, b2 = _build_w2b2(np.asarray(W, np.float32), np.asarray(b, np.float32))
    ksT = W2 @ np.asarray(lat, np.float32).T + b2[:, None]  # [KT2, 16]
    v = ksT.reshape(TILES, 128, B)  # [t, p, b]
    out = np.zeros((B, FOUT, HWP), np.float32)
    for bi in range(B):
        xb = np.concatenate([x[bi], np.ones((1, HWP), np.float32)], axis=0)  # [65, HWP]
        a_in = v[T_KIN : T_KIN + 128, 0:65, bi].T  # lhsT [65, 128]
        h1 = np.maximum(a_in.T @ xb, 0.0)
        a_ma = v[T_MIDA : T_MIDA + 128, :, bi].T  # [128, 128]
        bma = v[T_BMIDA, :, bi][:, None]
        h2 = np.maximum(a_ma.T @ h1 + bma, 0.0)
        a_mb = v[T_MIDB : T_MIDB + 128, :, bi].T
        bmb = v[T_BMIDB, :, bi][:, None]
        h3 = np.maximum(a_mb.T @ h2 + bmb, 0.0)
        a_out = v[T_KOUT : T_KOUT + 64, :, bi].T  # [128, 64]
        a_sh = v[T_KSH : T_KSH + 64, 0:65, bi].T  # [65, 64]
        out[bi] = a_out.T @ h3 + a_sh.T @ xb
    return out.reshape(B, FOUT, 128, 128)


# ---------------------------- bass program ----------------------------------

def _build_nc():
    import concourse.bass as bass
    import concourse.tile as tile
    from concourse import bacc, mybir

    F32 = mybir.dt.float32
    DT = mybir.dt.bfloat16 if DT_MODE == "bf16" else mybir.dt.float32r
    MMN = 512  # moving free-dim per matmul (PSUM bank limit)
    AF = mybir.ActivationFunctionType
    ALU = mybir.AluOpType

    nc = bacc.Bacc("TRN2", target_bir_lowering=False, debug=False, num_devices=NCORE)

    wmain = nc.dram_tensor("wmain", [128, TPC * 4 * 128], DT, kind="ExternalInput")
    latt = nc.dram_tensor("latt", [128, 64], DT, kind="ExternalInput")
    b2dup = nc.dram_tensor("b2dup", [128, TILES * 2], DT, kind="ExternalInput")
    xs = nc.dram_tensor("xs", [BC, FIN + 1, HWP], DT, kind="ExternalInput")
    outd = nc.dram_tensor("out", [BC, FOUT, HWP], F32, kind="ExternalOutput")

    with tile.TileContext(nc) as tc:
        with (
            tc.tile_pool(name="persist", bufs=1) as pp,
            tc.tile_pool(name="wpool", bufs=2) as wp,
            tc.tile_pool(name="conv", bufs=2) as cp,
            tc.tile_pool(name="ps", bufs=4, space="PSUM") as psp,
            tc.tile_pool(name="dram", bufs=1, space="DRAM") as dp,
        ):
            latt_sb = pp.tile([128, 64], DT)
            nc.sync.dma_start(latt_sb[:], latt[:])
            b2_sb = pp.tile([128, TILES * 2], DT)
            nc.sync.dma_start(b2_sb[:], b2dup[:])

            # ---- hypernet: ks tiles for this core's W shard, all 16 samples
            # ks_sh column order: d (dest core) major, then tile, then local sample
            ks_sh = pp.tile([128, TPC * 16], DT)
            ks_sh_v = ks_sh.rearrange("p (d t s) -> p d t s", d=NCORE, t=TPC, s=BC)
            wmv = wmain[:].rearrange("p (t l m) -> p t l m", t=TPC, l=4, m=128)
            for ch in range(NWCH):
                wt = wp.tile([128, WCH * 4 * 128], DT, tag="wt")
                nc.sync.dma_start(
                    wt[:], wmv[:, ch * WCH : (ch + 1) * WCH].rearrange("p t l m -> p (t l m)")
                )
                wtv = wt.rearrange("p (t l m) -> p t l m", t=WCH, l=4)
                pk = psp.tile([128, WCH * 16], F32, tag="ps")
                for tl in range(WCH):
                    for l in range(4):
                        nc.tensor.matmul(
                            pk[:, tl * 16 : (tl + 1) * 16],
                            wtv[:, tl, l, :],
                            latt_sb[:, l * 16 : (l + 1) * 16],
                            start=(l == 0),
                            stop=(l == 3),
                        )
                # permuting copy PSUM -> SBUF: col tl*16 + (d*2+s) -> (d, tile, s)
                pkv = pk.rearrange("p (t d s) -> p d t s", t=WCH, d=NCORE, s=BC)
                nc.vector.tensor_copy(
                    ks_sh_v[:, :, ch * WCH : (ch + 1) * WCH, :], pkv
                )

            # ---- AllToAll: core c receives the full kernel set for its samples
            cc_in = dp.tile([NCORE, 128, TPC * BC], DT)
            nc.sync.dma_start(
                cc_in[:].rearrange("d p r -> p d r"),
                ks_sh_v.rearrange("p d t s -> p d (t s)"),
            )
            cc_out = dp.tile([NCORE, 128, TPC * BC], DT)
            nc.gpsimd.collective_compute(
                "AllToAll",
                ALU.bypass,
                replica_groups=[list(range(NCORE))],
                ins=[cc_in.opt()],
                outs=[cc_out.opt()],
            )
            ksraw = pp.tile([128, TILES * BC], DT)
            nc.sync.dma_start(
                ksraw.rearrange("p (c r) -> p c r", c=NCORE),
                cc_out[:].rearrange("c p r -> p c r"),
            )
            ksall = pp.tile([128, TILES * BC], DT)
            nc.vector.tensor_tensor(ksall[:], ksraw[:], b2_sb[:], op=ALU.add)
            ksv = ksall.rearrange("p (t s) -> p t s", t=TILES, s=BC)
            # f32 per-partition bias vectors for the mid layers
            bias4 = pp.tile([128, 4], F32)
            nc.vector.tensor_copy(bias4[:, 0:2], ksall[:, T_BMIDA * 2 : T_BMIDA * 2 + 2])
            nc.vector.tensor_copy(bias4[:, 2:4], ksall[:, T_BMIDB * 2 : T_BMIDB * 2 + 2])

            # ---- conv: 2 samples x 8 pixel chunks
            ek = 0  # DVE/ACT alternation counter

            def relu_bias(dst, src, bias):
                nonlocal ek
                ek += 1
                if ek % 2 == 0:
                    if bias is None:
                        nc.vector.tensor_scalar_max(dst, src, 0.0)
                    else:
                        nc.vector.tensor_scalar(
                            dst, src, bias, 0.0, op0=ALU.add, op1=ALU.max
                        )
                else:
                    if bias is None:
                        nc.scalar.activation(dst, src, AF.Relu)
                    else:
                        nc.scalar.activation(dst, src, AF.Relu, bias=bias)

            def copy_out(dst, src):
                nonlocal ek
                ek += 1
                if ek % 2 == 0:
                    nc.vector.tensor_copy(dst, src)
                else:
                    nc.scalar.activation(dst, src, AF.Copy)

            def layer(dst_h, lhsT, rhs_tile, bias, kparts):
                for half in range(JP // NP):
                    ph = psp.tile([128, NP], F32, tag="ps", name=f"ph{half}")
                    for n in range(NP // MMN):
                        lo = half * NP + n * MMN
                        nc.tensor.matmul(
                            ph[:, n * MMN : (n + 1) * MMN],
                            lhsT,
                            rhs_tile[0:kparts, lo : lo + MMN],
                            start=True,
                            stop=True,
                        )
                    hsl = slice(half * NP, (half + 1) * NP)
                    relu_bias(dst_h[:, hsl], ph[:, 0:NP], bias)

            for j in range(NJ):
                jsl = slice(j * JP, (j + 1) * JP)
                xcs = []
                for s in range(BC):
                    xc = cp.tile([FIN + 1, JP], DT, tag="xc", bufs=4)
                    nc.scalar.dma_start(xc[:], xs[s, :, jsl])
                    xcs.append(xc)
                for s in range(BC):
                    xc = xcs[s]
                    h1 = cp.tile([128, JP], DT, tag="h", bufs=4)
                    layer(h1, ksv[0:65, T_KIN : T_KIN + 128, s], xc, None, 65)
                    h2 = cp.tile([128, JP], DT, tag="h", bufs=4)
                    layer(h2, ksv[:, T_MIDA : T_MIDA + 128, s], h1, bias4[:, s : s + 1], 128)
                    h3 = cp.tile([128, JP], DT, tag="h", bufs=4)
                    layer(h3, ksv[:, T_MIDB : T_MIDB + 128, s], h2, bias4[:, 2 + s : 3 + s], 128)

                    # out layer: k_out @ h3 + k_short' @ [x; 1]  (bias via ones row)
                    oc = cp.tile([64, JP], F32, tag="oc", bufs=4)
                    for half in range(JP // NP):
                        po = psp.tile([64, NP], F32, tag="ps", name=f"po{half}")
                        for n in range(NP // MMN):
                            lo = half * NP + n * MMN
                            nsl = slice(n * MMN, (n + 1) * MMN)
                            nc.tensor.matmul(
                                po[:, nsl],
                                ksv[:, T_KOUT : T_KOUT + 64, s],
                                h3[:, lo : lo + MMN],
                                start=True,
                                stop=False,
                            )
                            nc.tensor.matmul(
                                po[:, nsl],
                                ksv[0:65, T_KSH : T_KSH + 64, s],
                                xc[:, lo : lo + MMN],
                                start=False,
                                stop=True,
                            )
                        hsl = slice(half * NP, (half + 1) * NP)
                        copy_out(oc[:, hsl], po[:, 0:NP])
                    nc.scalar.dma_start(outd[s, :, jsl], oc[:])

    nc.compile()
    return nc


_NC_CACHE = None


def kernel(x, lat, W, b):
    from concourse.bass_utils import run_bass_kernel_spmd

    global _NC_CACHE
    if _NC_CACHE is None:
        _NC_CACHE = _build_nc()
    nc = _NC_CACHE
    in_maps = _host_inputs(x, lat, W, b)
    res = run_bass_kernel_spmd(nc, in_maps, core_ids=list(range(NCORE)))
    out = np.concatenate([res.results[c]["out"] for c in range(NCORE)], axis=0)
    return np.ascontiguousarray(out.reshape(B, FOUT, 128, 128))
